# revision 12
# baseline (speedup 1.0000x reference)
"""Trainium2 Bass kernel for nn_AttentionBasedIO.

The reference module computes, for each query position p:
    enc(p) @ keys.T  ->  softmax(./0.1)  ->  @ values
where keys[j] = binary_encoding(j) and enc(p) = binary_encoding(p).
Scores are 16 - 2*hamming(p, j): the argmax j == p wins the softmax by a
margin of e^-20 per hamming-1 neighbor, so the attention is one-hot to ~3e-8
relative.  With valid == ones the whole module is a row gather:
    out[i] = values[position[i]].

Strategy: data-parallel over the 16384 queries across 8 NeuronCores (2048
each); the [4096, 8] values table is replicated on every core (padded to
64 f32 per row because 256B is the SWDGE dma_gather element granularity).
Per core:

  1. Two HWDGE loads bring the pre-wrapped int16 index tile [128, 64+64]
     into SBUF (one per gather half, so gather 0 isn't gated on half 1).
  2. Two dma_gather calls (1024 idxs each - the Q7 ucode caps one call at
     1024 indices / 64 int16 per partition) pull 2048 rows of 256B into
     dst [128, 16, 64].  The host-side index layout is chosen so that
     dst[p, c, 0:8] = values[pos[p*16 + c]]  (dma_gather consumes wrapped
     index slot [s % 16, s // 16] for output slot s = c*128 + p).
  3. Two HWDGE stores write dst[:, half, 0:8] -> out DRAM [128, 128 f32]
     (output rows p*16 .. p*16+15 are contiguous per partition); store 0
     overlaps gather 1.

No Tile/Block wrapper: the five instructions + semaphores are hand-placed,
which avoids the Tile exit drain + all-engine barrier (~1us).
"""

import contextlib
import os
import sys

import numpy as np

for _p in ("/opt/trn_rl_repo",):
    if _p not in sys.path:
        sys.path.insert(0, _p)

import concourse.bacc as bacc
import concourse.mybir as mybir
from concourse.bass_utils import run_bass_kernel_spmd
from concourse.library_config import mlp

N_CORES = 8
BATCH = 16384
PER_CORE = BATCH // N_CORES  # 2048
P = 128
CH = PER_CORE // P  # 16 gathered rows per partition
V = 4096
D = 8
E = 64  # padded row: 64 f32 = 256B (dma_gather elem granularity)
NPG = 1024  # idxs per dma_gather call (ucode cap)
CPG = NPG // P  # 8 dst chunks per gather
ICOL = NPG // 16  # 64 idx-sbuf columns per gather

_CACHED_NC = None


def _build_nc():
    nc = bacc.Bacc("TRN2")
    idxs = nc.dram_tensor("idxs", [P, 2 * ICOL], mybir.dt.int16, kind="ExternalInput")
    vals = nc.dram_tensor("vals", [V, E], mybir.dt.float32, kind="ExternalInput")
    out = nc.dram_tensor("out", [P, CH * D], mybir.dt.float32, kind="ExternalOutput")

    with (
        nc.sbuf_tensor("idx_sb", [P, 2 * ICOL], mybir.dt.int16) as idx_sb,
        nc.sbuf_tensor("dst", [P, CH, E], mybir.dt.float32) as dst,
        contextlib.ExitStack() as st,
    ):
        s_idx = [st.enter_context(nc.semaphore(f"s_idx{i}")) for i in range(2)]
        s_g = [st.enter_context(nc.semaphore(f"s_g{i}")) for i in range(2)]
        s_o = [st.enter_context(nc.semaphore(f"s_o{i}")) for i in range(2)]

        gp, sp = nc.gpsimd, nc.sync
        gp.load_library(mlp)
        for i in range(2):
            sp.dma_start(
                out=idx_sb[:, i * ICOL : (i + 1) * ICOL],
                in_=idxs[:, i * ICOL : (i + 1) * ICOL],
            ).then_inc(s_idx[i], 16)
        for i in range(2):
            gp.wait_ge(s_idx[i], 16)
            gp.dma_gather(
                dst[:, i * CPG : (i + 1) * CPG, :],
                vals[:],
                idx_sb[:, i * ICOL : (i + 1) * ICOL],
                NPG,
                NPG,
                E,
            ).then_inc(s_g[i], 16)
        for i in range(2):
            sp.wait_ge(s_g[i], 16)
            sp.dma_start(
                out=out[:, i * CPG * D : (i + 1) * CPG * D],
                in_=dst[:, i * CPG : (i + 1) * CPG, :D],
            ).then_inc(s_o[i], 16)
        for i in range(2):
            sp.wait_ge(s_o[i], 16)

    nc.compile()
    return nc


# Host-side wrapped-index layout for one dma_gather call of NPG indices:
# output slot s (= c*128 + p) consumes wrapped[s % 16, s // 16]; we want
# slot (p, c) to receive pos[p*CPG + c], so wrapped[s % 16, s // 16] =
# pos[(s % 128)*CPG + s // 128].  The [16, 64] result is tiled to all 128
# partitions (the 8 GPSIMD cores each read their own 16-partition replica).
_S = np.arange(NPG)
_WRAP_PERM = np.empty(NPG, dtype=np.int64)
_WRAP_PERM[(_S % 16) * ICOL + _S // 16] = (_S % P) * CPG + _S // P

_CACHED_RUNNER = None


def _build_cached_runner(nc):
    """Cached analogue of bass2jax.run_bass_via_pjrt's multi-core branch: the
    jitted shard_map is built once, so warm kernel() calls skip jax re-tracing
    (run_bass_via_pjrt builds a fresh closure per call and re-traces)."""
    import jax
    from jax.sharding import Mesh, PartitionSpec
    from jax.experimental.shard_map import shard_map

    from concourse import bass2jax, mybir as _mybir

    bass2jax.install_neuronx_cc_hook()

    partition_name = nc.partition_id_tensor.name if nc.partition_id_tensor else None
    in_names, out_names, out_avals, out_shapes = [], [], [], []
    for alloc in nc.m.functions[0].allocations:
        if not isinstance(alloc, _mybir.MemoryLocationSet):
            continue
        name = alloc.memorylocations[0].name
        if alloc.kind == "ExternalInput":
            if name != partition_name:
                in_names.append(name)
        elif alloc.kind == "ExternalOutput":
            shape = tuple(alloc.tensor_shape)
            dtype = _mybir.dt.np(alloc.dtype)
            out_names.append(name)
            out_shapes.append((shape, dtype))
            out_avals.append(jax.core.ShapedArray(shape, dtype))
    n_params = len(in_names)
    all_names = tuple(
        in_names + out_names + ([partition_name] if partition_name else [])
    )

    def _body(*args):
        operands = list(args)
        if partition_name is not None:
            operands.append(bass2jax.partition_id_tensor())
        outs = bass2jax._bass_exec_p.bind(
            *operands,
            out_avals=tuple(out_avals),
            in_names=all_names,
            out_names=tuple(out_names),
            lowering_input_output_aliases=(),
            sim_require_finite=True,
            sim_require_nnan=True,
            nc=nc,
        )
        return tuple(outs)

    devices = jax.devices()[:N_CORES]
    mesh = Mesh(np.asarray(devices), ("core",))
    n_args = n_params + len(out_names)
    # "vals" is identical on every core: use a replicated spec so one copy is
    # shipped/broadcast instead of an 8x concat.
    in_specs = tuple(
        PartitionSpec() if nm == "vals" else PartitionSpec("core")
        for nm in in_names
    ) + (PartitionSpec("core"),) * len(out_names)
    sharded = jax.jit(
        shard_map(
            _body,
            mesh=mesh,
            in_specs=in_specs,
            out_specs=(PartitionSpec("core"),) * len(out_names),
            check_rep=False,
        ),
        donate_argnums=tuple(range(n_params, n_args)),
        keep_unused=True,
    )

    def run(in_maps):
        concat_in = [
            in_maps[0][nm]
            if nm == "vals"
            else np.concatenate([in_maps[c][nm] for c in range(N_CORES)], axis=0)
            for nm in in_names
        ]
        concat_zeros = [
            np.zeros((N_CORES * s[0], *s[1:]), dt) for s, dt in out_shapes
        ]
        out_arrs = sharded(*concat_in, *concat_zeros)
        return [
            {
                nm: np.asarray(out_arrs[i]).reshape(N_CORES, *out_shapes[i][0])[c]
                for i, nm in enumerate(out_names)
            }
            for c in range(N_CORES)
        ]

    return run


def kernel(position, keys, values, valid, _want_results=False):
    global _CACHED_NC
    del keys  # keys[j] is binary_encoding(j) by construction; not needed

    position = np.asarray(position)
    values = np.ascontiguousarray(np.asarray(values, dtype=np.float32))
    assert position.shape == (BATCH,)
    assert values.shape == (V, D)

    valid = np.asarray(valid)
    if not np.all(valid == 1.0):
        # Out-of-spec input (setup_inputs always passes ones): masked slots
        # change the softmax winner, so fall back to the exact reference math.
        bits = (position[:, None].astype(np.int64) >> np.arange(16)) & 1
        q = 2.0 * bits.astype(np.float32) - 1.0
        kb = (np.arange(V)[:, None] >> np.arange(16)) & 1
        k = 2.0 * kb.astype(np.float32) - 1.0
        scores = q @ k.T + (1.0 - valid[None, :]) * -1.0e9
        scores = scores / 0.1
        scores -= scores.max(axis=-1, keepdims=True)
        e = np.exp(scores)
        attn = e / e.sum(axis=-1, keepdims=True)
        return (attn @ values).astype(np.float32)

    vals_pad = np.zeros((V, E), dtype=np.float32)
    vals_pad[:, :D] = values

    if _CACHED_NC is None:
        _CACHED_NC = _build_nc()
    nc = _CACHED_NC

    pos16 = position.astype(np.int16)
    in_maps = []
    for c in range(N_CORES):
        chunk = pos16[c * PER_CORE : (c + 1) * PER_CORE].reshape(P, CH)
        # gather 0 covers chunk columns 0:8, gather 1 covers columns 8:16
        w0 = chunk[:, :CPG].ravel()[_WRAP_PERM].reshape(16, ICOL)
        w1 = chunk[:, CPG:].ravel()[_WRAP_PERM].reshape(16, ICOL)
        idxs = np.tile(np.concatenate([w0, w1], axis=1), (8, 1))
        in_maps.append({"idxs": idxs, "vals": vals_pad})

    global _CACHED_RUNNER
    if _CACHED_RUNNER is None:
        # First call goes through bass_utils.run_bass_kernel_spmd (compiles
        # the NEFF, runs on cores 0-7, and produces the NTFF profile when the
        # environment supports tracing).
        try:
            res = run_bass_kernel_spmd(nc, in_maps, core_ids=list(range(N_CORES)))
        except ModuleNotFoundError as e:
            # BASS_TRACE in an environment without the axon NTFF profile hook
            # (antenv.axon_hooks) would crash inside run_bass_kernel_spmd;
            # fall back to an untraced run.
            if "antenv" not in str(e):
                raise
            os.environ["BASS_NEVER_TRACE"] = "1"
            res = run_bass_kernel_spmd(nc, in_maps, core_ids=list(range(N_CORES)))
        _CACHED_RUNNER = _build_cached_runner(nc)
    else:
        # Warm path: reuse the jitted executable (no jax re-trace).
        from concourse.bass_utils import BassKernelResults

        res = BassKernelResults(
            results=_CACHED_RUNNER(in_maps),
            instructions_and_trace=None,
            profile_json=None,
            exec_time_ns=None,
        )

    out = np.concatenate(
        [res.results[c]["out"].reshape(PER_CORE, D) for c in range(N_CORES)], axis=0
    )
    if _want_results:
        return out, res
    return out


# revision 13
# speedup vs baseline: 1.1252x; 1.1252x over previous
"""Trainium2 Bass kernel for nn_AttentionBasedIO.

The reference module computes, for each query position p:
    enc(p) @ keys.T  ->  softmax(./0.1)  ->  @ values
where keys[j] = binary_encoding(j) and enc(p) = binary_encoding(p).
Scores are 16 - 2*hamming(p, j): the argmax j == p wins the softmax by a
margin of e^-20 per hamming-1 neighbor, so the attention is one-hot to ~3e-8
relative.  With valid == ones the whole module is a row gather:
    out[i] = values[position[i]].

Strategy: data-parallel over the 16384 queries across 8 NeuronCores (2048
each); the [4096, 8] values table is replicated on every core (padded to
64 f32 per row because 256B is the SWDGE dma_gather element granularity).
Per core:

  1. HWDGE loads bring the pre-wrapped int16 index tile into SBUF (one DMA
     per gather half plus one for the constant scatter block indices).
  2. Two dma_gather calls (1024 idxs each - the Q7 ucode caps one call at
     1024 indices / 64 int16 per partition) pull 2048 rows of 256B into
     dst [128, 16, 64].  The host-side index layout is chosen so that
     dst[p, c, 0:8] = values[pos[p*16 + c]]  (dma_gather consumes wrapped
     index slot [s % 16, s // 16] for output slot s = c*128 + p).
  3. While the gathers run, two dma_scatter_add stores are PREPARED
     (prepare_only=True: Q7 generates their descriptors up front - they
     depend only on addresses).  As each gather half lands, the DVE
     compacts it ([128, 8, 0:8 of 64] -> [128, 64], stripping the 256B
     padding) and a tiny trigger_dma fires the prepared scatter, which
     writes the half as 128 blocks of 256B onto the pre-zeroed output
     (+= on zeros == store).  The triggered transfer skips the ~1.3us
     HWDGE + DGE-delay issue chain a normal store would pay after the
     last gather's semaphore, cutting the critical-path tail.

Output is declared as [256, 64] f32 (256 blocks of 8 rows x 8 f32); the
host reshapes to [2048, 8].  No Tile/Block wrapper: instructions and
semaphores are hand-placed (Tile's exit drain also trips a walrus 2-wait
limit on this toolchain).
"""

import contextlib
import os
import sys

import numpy as np

for _p in ("/opt/trn_rl_repo",):
    if _p not in sys.path:
        sys.path.insert(0, _p)

import concourse.bacc as bacc
import concourse.mybir as mybir
from concourse.bass_utils import run_bass_kernel_spmd
from concourse.library_config import mlp

N_CORES = 8
BATCH = 16384
PER_CORE = BATCH // N_CORES  # 2048
P = 128
CH = PER_CORE // P  # 16 gathered rows per partition
V = 4096
D = 8
E = 64  # padded row: 64 f32 = 256B (dma_gather elem granularity)
NPG = 1024  # idxs per dma_gather call (ucode cap)
CPG = NPG // P  # 8 dst chunks per gather
ICOL = NPG // 16  # 64 idx-sbuf columns per gather
NBLK = 128  # 256B output blocks per scatter half (1024 rows / 8)
SCOL = NBLK // 16  # 8 idx-sbuf columns per scatter

_CACHED_NC = None


def _build_nc():
    nc = bacc.Bacc("TRN2")
    idxs = nc.dram_tensor(
        "idxs", [P, 2 * ICOL + 2 * SCOL], mybir.dt.int16, kind="ExternalInput"
    )
    vals = nc.dram_tensor("vals", [V, E], mybir.dt.float32, kind="ExternalInput")
    out = nc.dram_tensor("out", [2 * NBLK, E], mybir.dt.float32, kind="ExternalOutput")

    with (
        nc.sbuf_tensor("idx_sb", [P, 2 * ICOL + 2 * SCOL], mybir.dt.int16) as idx_sb,
        nc.sbuf_tensor("dst", [P, CH, E], mybir.dt.float32) as dst,
        nc.sbuf_tensor("cmp", [P, 2, E], mybir.dt.float32) as cmp,
        contextlib.ExitStack() as st,
    ):
        s_idx = [st.enter_context(nc.semaphore(f"s_idx{i}")) for i in range(3)]
        s_g = [st.enter_context(nc.semaphore(f"s_g{i}")) for i in range(2)]
        s_c = [st.enter_context(nc.semaphore(f"s_c{i}")) for i in range(2)]
        s_p = [st.enter_context(nc.semaphore(f"s_p{i}")) for i in range(2)]
        s_o = [st.enter_context(nc.semaphore(f"s_o{i}")) for i in range(2)]

        gp, sp, dv = nc.gpsimd, nc.sync, nc.vector
        gp.load_library(mlp)
        for i in range(2):
            sp.dma_start(
                out=idx_sb[:, i * ICOL : (i + 1) * ICOL],
                in_=idxs[:, i * ICOL : (i + 1) * ICOL],
            ).then_inc(s_idx[i], 16)
        sp.dma_start(
            out=idx_sb[:, 2 * ICOL :], in_=idxs[:, 2 * ICOL :]
        ).then_inc(s_idx[2], 16)

        for i in range(2):
            gp.wait_ge(s_idx[i], 16)
            gp.dma_gather(
                dst[:, i * CPG : (i + 1) * CPG, :],
                vals[:],
                idx_sb[:, i * ICOL : (i + 1) * ICOL],
                NPG,
                NPG,
                E,
            ).then_inc(s_g[i], 16)

        # Prepare both scatter stores while the gathers' transfers run.
        gp.wait_ge(s_idx[2], 16)
        for i in range(2):
            gp.dma_scatter_add(
                out[:],
                cmp[:, i : i + 1, :],
                idx_sb[
                    :, 2 * ICOL + i * SCOL : 2 * ICOL + (i + 1) * SCOL
                ],
                NBLK,
                NBLK,
                E,
                prepare_only=True,
                sem=s_o[i],
            ).then_inc(s_p[i], 1)

        # DVE compaction per half: dst[:, half, 0:8 of 64] -> cmp[:, half, :]
        for i in range(2):
            dv.wait_ge(s_g[i], 16)
            dv.tensor_copy(
                out=cmp[:, i : i + 1, :].rearrange("p one (c d) -> p (one c) d", d=D),
                in_=dst[:, i * CPG : (i + 1) * CPG, :D],
            ).then_inc(s_c[i], 1)

        # Fire each prepared scatter as soon as its half is compacted.
        for i in range(2):
            gp.wait_ge(s_p[i], 1)
            gp.wait_ge(s_c[i], 1)
            gp.trigger_dma(count=1)
        for i in range(2):
            gp.wait_ge(s_o[i], 16)

    nc.compile()
    return nc


# Host-side wrapped-index layout for one dma_gather call of NPG indices:
# output slot s (= c*128 + p) consumes wrapped[s % 16, s // 16]; we want
# slot (p, c) to receive pos[p*CPG + c], so wrapped[s % 16, s // 16] =
# pos[(s % 128)*CPG + s // 128].  The [16, 64] result is tiled to all 128
# partitions (the 8 GPSIMD cores each read their own 16-partition replica).
_S = np.arange(NPG)
_WRAP_PERM = np.empty(NPG, dtype=np.int64)
_WRAP_PERM[(_S % 16) * ICOL + _S // 16] = (_S % P) * CPG + _S // P


def _scatter_idx(h):
    # scatter src slot s = p (single chunk) holds output rows
    # p*16 + h*8 .. p*16 + h*8 + 7 = 256B block 2p + h
    w = np.empty((16, SCOL), dtype=np.int16)
    s = np.arange(NBLK)
    w[s % 16, s // 16] = (2 * s + h).astype(np.int16)
    return w


_SIDX = np.tile(np.concatenate([_scatter_idx(0), _scatter_idx(1)], axis=1), (8, 1))

_CACHED_RUNNER = None


def _build_cached_runner(nc):
    """Cached analogue of bass2jax.run_bass_via_pjrt's multi-core branch: the
    jitted shard_map is built once, so warm kernel() calls skip jax re-tracing
    (run_bass_via_pjrt builds a fresh closure per call and re-traces)."""
    import jax
    from jax.sharding import Mesh, PartitionSpec
    from jax.experimental.shard_map import shard_map

    from concourse import bass2jax, mybir as _mybir

    bass2jax.install_neuronx_cc_hook()

    partition_name = nc.partition_id_tensor.name if nc.partition_id_tensor else None
    in_names, out_names, out_avals, out_shapes = [], [], [], []
    for alloc in nc.m.functions[0].allocations:
        if not isinstance(alloc, _mybir.MemoryLocationSet):
            continue
        name = alloc.memorylocations[0].name
        if alloc.kind == "ExternalInput":
            if name != partition_name:
                in_names.append(name)
        elif alloc.kind == "ExternalOutput":
            shape = tuple(alloc.tensor_shape)
            dtype = _mybir.dt.np(alloc.dtype)
            out_names.append(name)
            out_shapes.append((shape, dtype))
            out_avals.append(jax.core.ShapedArray(shape, dtype))
    n_params = len(in_names)
    all_names = tuple(
        in_names + out_names + ([partition_name] if partition_name else [])
    )

    def _body(*args):
        operands = list(args)
        if partition_name is not None:
            operands.append(bass2jax.partition_id_tensor())
        outs = bass2jax._bass_exec_p.bind(
            *operands,
            out_avals=tuple(out_avals),
            in_names=all_names,
            out_names=tuple(out_names),
            lowering_input_output_aliases=(),
            sim_require_finite=True,
            sim_require_nnan=True,
            nc=nc,
        )
        return tuple(outs)

    devices = jax.devices()[:N_CORES]
    mesh = Mesh(np.asarray(devices), ("core",))
    n_args = n_params + len(out_names)
    # "vals" is identical on every core: use a replicated spec so one copy is
    # shipped/broadcast instead of an 8x concat.
    in_specs = tuple(
        PartitionSpec() if nm == "vals" else PartitionSpec("core")
        for nm in in_names
    ) + (PartitionSpec("core"),) * len(out_names)
    sharded = jax.jit(
        shard_map(
            _body,
            mesh=mesh,
            in_specs=in_specs,
            out_specs=(PartitionSpec("core"),) * len(out_names),
            check_rep=False,
        ),
        donate_argnums=tuple(range(n_params, n_args)),
        keep_unused=True,
    )

    def run(in_maps):
        concat_in = [
            in_maps[0][nm]
            if nm == "vals"
            else np.concatenate([in_maps[c][nm] for c in range(N_CORES)], axis=0)
            for nm in in_names
        ]
        concat_zeros = [
            np.zeros((N_CORES * s[0], *s[1:]), dt) for s, dt in out_shapes
        ]
        out_arrs = sharded(*concat_in, *concat_zeros)
        return [
            {
                nm: np.asarray(out_arrs[i]).reshape(N_CORES, *out_shapes[i][0])[c]
                for i, nm in enumerate(out_names)
            }
            for c in range(N_CORES)
        ]

    return run


def kernel(position, keys, values, valid, _want_results=False):
    global _CACHED_NC
    del keys  # keys[j] is binary_encoding(j) by construction; not needed

    position = np.asarray(position)
    values = np.ascontiguousarray(np.asarray(values, dtype=np.float32))
    assert position.shape == (BATCH,)
    assert values.shape == (V, D)

    valid = np.asarray(valid)
    if not np.all(valid == 1.0):
        # Out-of-spec input (setup_inputs always passes ones): masked slots
        # change the softmax winner, so fall back to the exact reference math.
        bits = (position[:, None].astype(np.int64) >> np.arange(16)) & 1
        q = 2.0 * bits.astype(np.float32) - 1.0
        kb = (np.arange(V)[:, None] >> np.arange(16)) & 1
        k = 2.0 * kb.astype(np.float32) - 1.0
        scores = q @ k.T + (1.0 - valid[None, :]) * -1.0e9
        scores = scores / 0.1
        scores -= scores.max(axis=-1, keepdims=True)
        e = np.exp(scores)
        attn = e / e.sum(axis=-1, keepdims=True)
        return (attn @ values).astype(np.float32)

    vals_pad = np.zeros((V, E), dtype=np.float32)
    vals_pad[:, :D] = values

    if _CACHED_NC is None:
        _CACHED_NC = _build_nc()
    nc = _CACHED_NC

    pos16 = position.astype(np.int16)
    in_maps = []
    for c in range(N_CORES):
        chunk = pos16[c * PER_CORE : (c + 1) * PER_CORE].reshape(P, CH)
        # gather 0 covers chunk columns 0:8, gather 1 covers columns 8:16
        w0 = chunk[:, :CPG].ravel()[_WRAP_PERM].reshape(16, ICOL)
        w1 = chunk[:, CPG:].ravel()[_WRAP_PERM].reshape(16, ICOL)
        gidx = np.tile(np.concatenate([w0, w1], axis=1), (8, 1))
        in_maps.append(
            {"idxs": np.concatenate([gidx, _SIDX], axis=1), "vals": vals_pad}
        )

    global _CACHED_RUNNER
    if _CACHED_RUNNER is None:
        # First call goes through bass_utils.run_bass_kernel_spmd (compiles
        # the NEFF, runs on cores 0-7, and produces the NTFF profile when the
        # environment supports tracing).
        try:
            res = run_bass_kernel_spmd(nc, in_maps, core_ids=list(range(N_CORES)))
        except ModuleNotFoundError as e:
            # BASS_TRACE in an environment without the axon NTFF profile hook
            # (antenv.axon_hooks) would crash inside run_bass_kernel_spmd;
            # fall back to an untraced run.
            if "antenv" not in str(e):
                raise
            os.environ["BASS_NEVER_TRACE"] = "1"
            res = run_bass_kernel_spmd(nc, in_maps, core_ids=list(range(N_CORES)))
        _CACHED_RUNNER = _build_cached_runner(nc)
    else:
        # Warm path: reuse the jitted executable (no jax re-trace).
        from concourse.bass_utils import BassKernelResults

        res = BassKernelResults(
            results=_CACHED_RUNNER(in_maps),
            instructions_and_trace=None,
            profile_json=None,
            exec_time_ns=None,
        )

    out = np.concatenate(
        [res.results[c]["out"].reshape(PER_CORE, D) for c in range(N_CORES)], axis=0
    )
    if _want_results:
        return out, res
    return out


# revision 14
# speedup vs baseline: 1.1919x; 1.0593x over previous
"""Trainium2 Bass kernel for nn_AttentionBasedIO.

The reference module computes, for each query position p:
    enc(p) @ keys.T  ->  softmax(./0.1)  ->  @ values
where keys[j] = binary_encoding(j) and enc(p) = binary_encoding(p).
Scores are 16 - 2*hamming(p, j): the argmax j == p wins the softmax by a
margin of e^-20 per hamming-1 neighbor, so the attention is one-hot to ~3e-8
relative.  With valid == ones the whole module is a row gather:
    out[i] = values[position[i]].

Strategy: data-parallel over the 16384 queries across 8 NeuronCores (2048
each); the [4096, 8] values table is replicated on every core (padded to
64 f32 per row because 256B is the SWDGE dma_gather element granularity).
Per core:

  1. HWDGE loads bring the pre-wrapped int16 index tile into SBUF (one DMA
     per gather half plus one for the constant scatter block indices).
  2. Two dma_gather calls (1024 idxs each - the Q7 ucode caps one call at
     1024 indices / 64 int16 per partition) pull 2048 rows of 256B into
     dst [128, 16, 64].  The host-side index layout is chosen so that
     dst[p, c, 0:8] = values[pos[p*16 + c]]  (dma_gather consumes wrapped
     index slot [s % 16, s // 16] for output slot s = c*128 + p).
  3. While the gathers run, two dma_scatter_add stores are PREPARED
     (prepare_only=True: Q7 generates their descriptors up front - they
     depend only on addresses).  As each gather half lands, the DVE
     compacts it ([128, 8, 0:8 of 64] -> [128, 64], stripping the 256B
     padding) and a tiny trigger_dma fires the prepared scatter, which
     writes the half as 128 blocks of 256B onto the pre-zeroed output
     (+= on zeros == store).  The triggered transfer skips the ~1.3us
     HWDGE + DGE-delay issue chain a normal store would pay after the
     last gather's semaphore, cutting the critical-path tail.

Output is declared as [256, 64] f32 (256 blocks of 8 rows x 8 f32); the
host reshapes to [2048, 8].  No Tile/Block wrapper: instructions and
semaphores are hand-placed (Tile's exit drain also trips a walrus 2-wait
limit on this toolchain).
"""

import contextlib
import os
import sys

import numpy as np

for _p in ("/opt/trn_rl_repo",):
    if _p not in sys.path:
        sys.path.insert(0, _p)

import concourse.bacc as bacc
import concourse.mybir as mybir
from concourse.bass_utils import run_bass_kernel_spmd
from concourse.library_config import mlp

N_CORES = 8
BATCH = 16384
PER_CORE = BATCH // N_CORES  # 2048
P = 128
CH = PER_CORE // P  # 16 gathered rows per partition
V = 4096
D = 8
E = 64  # padded row: 64 f32 = 256B (dma_gather elem granularity)
NPG = 1024  # idxs per dma_gather call (ucode cap)
CPG = NPG // P  # 8 dst chunks per gather
ICOL = NPG // 16  # 64 idx-sbuf columns per gather
NBLK = 128  # 256B output blocks per scatter half (1024 rows / 8)
SCOL = NBLK // 16  # 8 idx-sbuf columns per scatter

_CACHED_NC = None


def _build_nc():
    nc = bacc.Bacc("TRN2")
    idxs = nc.dram_tensor(
        "idxs", [P, 2 * ICOL + 2 * SCOL], mybir.dt.int16, kind="ExternalInput"
    )
    vals = nc.dram_tensor("vals", [V, E], mybir.dt.float32, kind="ExternalInput")
    out = nc.dram_tensor("out", [2 * NBLK, E], mybir.dt.float32, kind="ExternalOutput")

    with (
        nc.sbuf_tensor("idx_sb", [P, 2 * ICOL + 2 * SCOL], mybir.dt.int16) as idx_sb,
        nc.sbuf_tensor("dst", [P, CH, E], mybir.dt.float32) as dst,
        nc.sbuf_tensor("cmp", [P, 2, E], mybir.dt.float32) as cmp,
        contextlib.ExitStack() as st,
    ):
        s_idx = [st.enter_context(nc.semaphore(f"s_idx{i}")) for i in range(3)]
        s_g = [st.enter_context(nc.semaphore(f"s_g{i}")) for i in range(2)]
        s_gp = [st.enter_context(nc.semaphore(f"s_gp{i}")) for i in range(2)]
        s_c = [st.enter_context(nc.semaphore(f"s_c{i}")) for i in range(2)]
        s_p = [st.enter_context(nc.semaphore(f"s_p{i}")) for i in range(2)]
        s_o = [st.enter_context(nc.semaphore(f"s_o{i}")) for i in range(2)]

        gp, sp, dv = nc.gpsimd, nc.sync, nc.vector
        gp.load_library(mlp)
        for i in range(2):
            sp.dma_start(
                out=idx_sb[:, i * ICOL : (i + 1) * ICOL],
                in_=idxs[:, i * ICOL : (i + 1) * ICOL],
            ).then_inc(s_idx[i], 16)
        sp.dma_start(
            out=idx_sb[:, 2 * ICOL :], in_=idxs[:, 2 * ICOL :]
        ).then_inc(s_idx[2], 16)

        # Gathers are also prepared + trigger-fired: the triggered transfer
        # starts at trigger dispatch, skipping the DGE doorbell delay a
        # self-firing SWDGE DMA pays between desc-gen and transfer start.
        for i in range(2):
            gp.wait_ge(s_idx[i], 16)
            gp.dma_gather(
                dst[:, i * CPG : (i + 1) * CPG, :],
                vals[:],
                idx_sb[:, i * ICOL : (i + 1) * ICOL],
                NPG,
                NPG,
                E,
                prepare_only=True,
                sem=s_g[i],
            ).then_inc(s_gp[i], 1)
            gp.wait_ge(s_gp[i], 1)
            gp.trigger_dma(count=1)

        # Prepare both scatter stores while the gathers' transfers run.
        gp.wait_ge(s_idx[2], 16)
        for i in range(2):
            gp.dma_scatter_add(
                out[:],
                cmp[:, i : i + 1, :],
                idx_sb[
                    :, 2 * ICOL + i * SCOL : 2 * ICOL + (i + 1) * SCOL
                ],
                NBLK,
                NBLK,
                E,
                prepare_only=True,
                sem=s_o[i],
            ).then_inc(s_p[i], 1)

        # DVE compaction per half: dst[:, half, 0:8 of 64] -> cmp[:, half, :]
        for i in range(2):
            dv.wait_ge(s_g[i], 16)
            dv.tensor_copy(
                out=cmp[:, i : i + 1, :].rearrange("p one (c d) -> p (one c) d", d=D),
                in_=dst[:, i * CPG : (i + 1) * CPG, :D],
            ).then_inc(s_c[i], 1)

        # Fire each prepared scatter as soon as its half is compacted.
        for i in range(2):
            gp.wait_ge(s_p[i], 1)
            gp.wait_ge(s_c[i], 1)
            gp.trigger_dma(count=1)
        for i in range(2):
            gp.wait_ge(s_o[i], 16)

    nc.compile()
    return nc


# Host-side wrapped-index layout for one dma_gather call of NPG indices:
# output slot s (= c*128 + p) consumes wrapped[s % 16, s // 16]; we want
# slot (p, c) to receive pos[p*CPG + c], so wrapped[s % 16, s // 16] =
# pos[(s % 128)*CPG + s // 128].  The [16, 64] result is tiled to all 128
# partitions (the 8 GPSIMD cores each read their own 16-partition replica).
_S = np.arange(NPG)
_WRAP_PERM = np.empty(NPG, dtype=np.int64)
_WRAP_PERM[(_S % 16) * ICOL + _S // 16] = (_S % P) * CPG + _S // P


def _scatter_idx(h):
    # scatter src slot s = p (single chunk) holds output rows
    # p*16 + h*8 .. p*16 + h*8 + 7 = 256B block 2p + h
    w = np.empty((16, SCOL), dtype=np.int16)
    s = np.arange(NBLK)
    w[s % 16, s // 16] = (2 * s + h).astype(np.int16)
    return w


_SIDX = np.tile(np.concatenate([_scatter_idx(0), _scatter_idx(1)], axis=1), (8, 1))

_CACHED_RUNNER = None


def _build_cached_runner(nc):
    """Cached analogue of bass2jax.run_bass_via_pjrt's multi-core branch: the
    jitted shard_map is built once, so warm kernel() calls skip jax re-tracing
    (run_bass_via_pjrt builds a fresh closure per call and re-traces)."""
    import jax
    from jax.sharding import Mesh, PartitionSpec
    from jax.experimental.shard_map import shard_map

    from concourse import bass2jax, mybir as _mybir

    bass2jax.install_neuronx_cc_hook()

    partition_name = nc.partition_id_tensor.name if nc.partition_id_tensor else None
    in_names, out_names, out_avals, out_shapes = [], [], [], []
    for alloc in nc.m.functions[0].allocations:
        if not isinstance(alloc, _mybir.MemoryLocationSet):
            continue
        name = alloc.memorylocations[0].name
        if alloc.kind == "ExternalInput":
            if name != partition_name:
                in_names.append(name)
        elif alloc.kind == "ExternalOutput":
            shape = tuple(alloc.tensor_shape)
            dtype = _mybir.dt.np(alloc.dtype)
            out_names.append(name)
            out_shapes.append((shape, dtype))
            out_avals.append(jax.core.ShapedArray(shape, dtype))
    n_params = len(in_names)
    all_names = tuple(
        in_names + out_names + ([partition_name] if partition_name else [])
    )

    def _body(*args):
        operands = list(args)
        if partition_name is not None:
            operands.append(bass2jax.partition_id_tensor())
        outs = bass2jax._bass_exec_p.bind(
            *operands,
            out_avals=tuple(out_avals),
            in_names=all_names,
            out_names=tuple(out_names),
            lowering_input_output_aliases=(),
            sim_require_finite=True,
            sim_require_nnan=True,
            nc=nc,
        )
        return tuple(outs)

    devices = jax.devices()[:N_CORES]
    mesh = Mesh(np.asarray(devices), ("core",))
    n_args = n_params + len(out_names)
    # "vals" is identical on every core: use a replicated spec so one copy is
    # shipped/broadcast instead of an 8x concat.
    in_specs = tuple(
        PartitionSpec() if nm == "vals" else PartitionSpec("core")
        for nm in in_names
    ) + (PartitionSpec("core"),) * len(out_names)
    sharded = jax.jit(
        shard_map(
            _body,
            mesh=mesh,
            in_specs=in_specs,
            out_specs=(PartitionSpec("core"),) * len(out_names),
            check_rep=False,
        ),
        donate_argnums=tuple(range(n_params, n_args)),
        keep_unused=True,
    )

    def run(in_maps):
        concat_in = [
            in_maps[0][nm]
            if nm == "vals"
            else np.concatenate([in_maps[c][nm] for c in range(N_CORES)], axis=0)
            for nm in in_names
        ]
        concat_zeros = [
            np.zeros((N_CORES * s[0], *s[1:]), dt) for s, dt in out_shapes
        ]
        out_arrs = sharded(*concat_in, *concat_zeros)
        return [
            {
                nm: np.asarray(out_arrs[i]).reshape(N_CORES, *out_shapes[i][0])[c]
                for i, nm in enumerate(out_names)
            }
            for c in range(N_CORES)
        ]

    return run


def kernel(position, keys, values, valid, _want_results=False):
    global _CACHED_NC
    del keys  # keys[j] is binary_encoding(j) by construction; not needed

    position = np.asarray(position)
    values = np.ascontiguousarray(np.asarray(values, dtype=np.float32))
    assert position.shape == (BATCH,)
    assert values.shape == (V, D)

    valid = np.asarray(valid)
    if not np.all(valid == 1.0):
        # Out-of-spec input (setup_inputs always passes ones): masked slots
        # change the softmax winner, so fall back to the exact reference math.
        bits = (position[:, None].astype(np.int64) >> np.arange(16)) & 1
        q = 2.0 * bits.astype(np.float32) - 1.0
        kb = (np.arange(V)[:, None] >> np.arange(16)) & 1
        k = 2.0 * kb.astype(np.float32) - 1.0
        scores = q @ k.T + (1.0 - valid[None, :]) * -1.0e9
        scores = scores / 0.1
        scores -= scores.max(axis=-1, keepdims=True)
        e = np.exp(scores)
        attn = e / e.sum(axis=-1, keepdims=True)
        return (attn @ values).astype(np.float32)

    vals_pad = np.zeros((V, E), dtype=np.float32)
    vals_pad[:, :D] = values

    if _CACHED_NC is None:
        _CACHED_NC = _build_nc()
    nc = _CACHED_NC

    pos16 = position.astype(np.int16)
    in_maps = []
    for c in range(N_CORES):
        chunk = pos16[c * PER_CORE : (c + 1) * PER_CORE].reshape(P, CH)
        # gather 0 covers chunk columns 0:8, gather 1 covers columns 8:16
        w0 = chunk[:, :CPG].ravel()[_WRAP_PERM].reshape(16, ICOL)
        w1 = chunk[:, CPG:].ravel()[_WRAP_PERM].reshape(16, ICOL)
        gidx = np.tile(np.concatenate([w0, w1], axis=1), (8, 1))
        in_maps.append(
            {"idxs": np.concatenate([gidx, _SIDX], axis=1), "vals": vals_pad}
        )

    global _CACHED_RUNNER
    if _CACHED_RUNNER is None:
        # First call goes through bass_utils.run_bass_kernel_spmd (compiles
        # the NEFF, runs on cores 0-7, and produces the NTFF profile when the
        # environment supports tracing).
        try:
            res = run_bass_kernel_spmd(nc, in_maps, core_ids=list(range(N_CORES)))
        except ModuleNotFoundError as e:
            # BASS_TRACE in an environment without the axon NTFF profile hook
            # (antenv.axon_hooks) would crash inside run_bass_kernel_spmd;
            # fall back to an untraced run.
            if "antenv" not in str(e):
                raise
            os.environ["BASS_NEVER_TRACE"] = "1"
            res = run_bass_kernel_spmd(nc, in_maps, core_ids=list(range(N_CORES)))
        _CACHED_RUNNER = _build_cached_runner(nc)
    else:
        # Warm path: reuse the jitted executable (no jax re-trace).
        from concourse.bass_utils import BassKernelResults

        res = BassKernelResults(
            results=_CACHED_RUNNER(in_maps),
            instructions_and_trace=None,
            profile_json=None,
            exec_time_ns=None,
        )

    out = np.concatenate(
        [res.results[c]["out"].reshape(PER_CORE, D) for c in range(N_CORES)], axis=0
    )
    if _want_results:
        return out, res
    return out
